# revision 34
# baseline (speedup 1.0000x reference)
"""Trainium2 Bass kernel for the Gaussian-span multi-head self-attention module.

  span  = head_reshape(h @ W_span.T, 2)          (B*K, M, 2)
  value = head_reshape(h @ W_val.T, D)           (B*K, M, D)
  mean  = sigmoid(span0) * M ; soft = softplus(span1)
  attn  = softmax(-soft * (pos - mean)^2)        (B*K, M, M)
  out   = (attn @ value)  -> concat heads -> @ W_out.T

Shapes are hardcoded: B=2, M=2048, HS=1024, K=16 heads, D=64.

Strategy (8 NeuronCores, SPMD — one program, per-core data):
  * batch*head sharding: core = b*4 + g handles batch b, heads [4g, 4g+4).
  * Host computes mean/soft (tiny span projection), sorts each head's rows by
    mean, and builds a windowed per-key-block schedule (envelope over all 32
    head instances so all cores share one NEFF).  ~8.5x fewer score elements
    than dense.
  * Scores: rank-3 matmul in float32r (1 cy/row at >=256 free size vs fp32's
    4 cy/row); per-block ranges padded to >=256 and consecutive blocks merged
    into <=512-wide matmuls.  One Exp activation per merged group.
  * attn @ value: value stationary is [64 ones cols | 64 value cols] per head,
    so the softmax denominator lands in PSUM partitions 0..63 PRE-BROADCAST
    (the custom-DVE reciprocal misreads PSUM at partition base 64, so ones go
    first); normalization is just DVE reciprocal + multiply per 512-col bank
    (no PE broadcast matmul, no single-partition copies).
  * Sorted->natural un-permutation via four gpsimd local_scatter ops emitted
    back-to-back at the end (consecutive emission lets the launches pack;
    separated gpsimd launches or indirect_copy eat ~28us each).
  * Output projection in bf16; per-core partials summed on host.
  * Inputs are host-packed so each tensor is a single contiguous DMA; hT is
    sliced per key block so the value matmul can start as blocks arrive.

Hardware notes baked into this design (verified by probes on this setup):
  - float32r matmul is TF32-grade (~12 mantissa bits): unusable for the score
    coefficients (|s*t^2| up to ~2e4 needs fp32), so scores stay fp32.
  - A matmul's start flag zeroes the full 2KB PSUM "zero region": PSUM tiles
    must not share a 2KB bank with live neighbors (pv lives in the shared
    2KB-slot 'sc' pool for this reason).
  - DVE ops tolerate input/output partition-base mismatches EXCEPT custom-DVE
    ucode (reciprocal_approx_fast) reading PSUM at a nonzero partition base.
"""

import sys
import types

import numpy as np
import ml_dtypes

B, M, HS, NH, D = 2, 2048, 1024, 16, 64
NCORES = 8
HPC = 4            # heads per core
CP = HPC * D       # 256-wide channel slice per core
NJB = M // 128     # key blocks
TAIL_T = 30.0      # window cut: exp(-TAIL_T) ~ 9e-14
MINW = 8           # score widths stay multiple-of-8 (no fp32r padding)
HM = M // 2

_CACHE = {}        # ranges tuple -> compiled Bass program
DEBUG_TAPS = False  # emit extra DMA-outs of intermediates for HW debugging


def _ensure_ntff_hook():
    """Install the antenv.axon_hooks shim if the image lacks it (profiling only)."""
    try:
        import antenv.axon_hooks  # noqa: F401
        return
    except ImportError:
        pass
    try:
        import antenv
        from trn_agent_boot.trn_boot import _ntff_profile_via_ctypes
    except ImportError:
        return
    mod = types.ModuleType("antenv.axon_hooks")
    _h = [None]
    mod.set_axon_ntff_profile_hook = lambda hk: _h.__setitem__(0, hk)
    mod.get_axon_ntff_profile_hook = lambda: _h[0]
    sys.modules["antenv.axon_hooks"] = mod
    antenv.axon_hooks = mod
    try:
        mod.set_axon_ntff_profile_hook(
            _ntff_profile_via_ctypes("/opt/axon/libaxon_pjrt.so"))
    except Exception:
        pass


def _sigmoid64(x):
    return 1.0 / (1.0 + np.exp(-x.astype(np.float64)))


def _softplus64(x):
    return np.logaddexp(0.0, x.astype(np.float64))


def _schedule(ranges):
    """Padded widths, offsets, and merged score-matmul groups."""
    wpads, offs_pad, cw_pad = [], [], 0
    for lo, hi in ranges:
        w = hi - lo
        wp = 0 if w == 0 else max(MINW, w)
        wpads.append(wp)
        offs_pad.append(cw_pad)
        cw_pad += wp
    groups = []  # list of (first_jb, njb, width)
    jb = 0
    while jb < NJB:
        if wpads[jb] == 0:
            jb += 1
            continue
        j2, wsum = jb, 0
        while j2 < NJB and wpads[j2] > 0 and wsum + wpads[j2] <= 512:
            wsum += wpads[j2]
            j2 += 1
        groups.append((jb, j2 - jb, wsum))
        jb = j2
    return wpads, offs_pad, cw_pad, groups


def _build_host_data(h, W_span, W_val, W_out):
    h = np.asarray(h, np.float32)
    W_span = np.asarray(W_span, np.float32)
    W_val = np.asarray(W_val, np.float32)
    W_out = np.asarray(W_out, np.float32)

    span = (h.reshape(B * M, HS) @ W_span.T).reshape(B, M, 2 * NH)

    m_all = np.zeros((B, NH, M), np.float64)
    s_all = np.zeros((B, NH, M), np.float64)
    for b in range(B):
        for k in range(NH):
            m_all[b, k] = _sigmoid64(span[b, :, 2 * k]) * M
            s_all[b, k] = _softplus64(span[b, :, 2 * k + 1])
    order_all = np.argsort(m_all, axis=-1, kind="stable")
    W_all = np.sqrt(TAIL_T / np.maximum(s_all, 1e-12))

    ilos = np.full(NJB, M, np.int64)
    ihis = np.zeros(NJB, np.int64)
    for b in range(B):
        for k in range(NH):
            ms = m_all[b, k][order_all[b, k]]
            ws = W_all[b, k][order_all[b, k]]
            lo, hi = ms - ws, ms + ws
            for jb in range(NJB):
                mask = (hi >= jb * 128) & (lo <= jb * 128 + 128)
                idx = np.flatnonzero(mask)
                if idx.size:
                    ilos[jb] = min(ilos[jb], idx[0])
                    ihis[jb] = max(ihis[jb], idx[-1] + 1)
    ranges = []
    for jb in range(NJB):
        if ihis[jb] <= ilos[jb]:
            ranges.append((0, 0))
        else:
            ranges.append((int(ilos[jb]) & ~7, min(M, (int(ihis[jb]) + 7) & ~7)))

    # coverage: every sorted row must fall in the range of its own mean's block
    for b in range(B):
        for k in range(NH):
            ms = m_all[b, k][order_all[b, k]]
            own = np.clip((ms // 128).astype(np.int64), 0, NJB - 1)
            pos = np.arange(M)
            lows = np.array([ranges[j][0] for j in own])
            highs = np.array([ranges[j][1] for j in own])
            if not ((lows <= pos) & (pos < highs)).all():
                raise AssertionError("window schedule does not cover all rows")

    wpads, offs_pad, cw_pad, _groups = _schedule(tuple(ranges))

    in_maps = []
    for core in range(NCORES):
        b, g = core // HPC, core % HPC
        heads = [g * HPC + kk for kk in range(HPC)]

        # hT packed: hTp[p, jb*1024 + c*128 + m] = h[b].T[c*128+p, jb*128+m]
        hbT = np.ascontiguousarray(h[b].T).astype(ml_dtypes.bfloat16)
        hTp = np.ascontiguousarray(
            hbT.reshape(8, 128, NJB, 128).transpose(1, 2, 0, 3).reshape(128, NJB * 1024))

        # Wv packed: Wvp[p, c*256 + o] = W_val[g*256+o, c*128+p]
        WvT = W_val[g * CP:(g + 1) * CP, :].T.astype(ml_dtypes.bfloat16)  # (HS, CP)
        Wvp = np.ascontiguousarray(WvT.reshape(8, 128, CP).transpose(1, 0, 2).reshape(128, 8 * CP))

        # Wo packed: Wop[p, p2*1024 + o] = W_out[o, g*256 + p2*128 + p]
        WoT = W_out[:, g * CP:(g + 1) * CP].T.astype(ml_dtypes.bfloat16)  # (CP, HS)
        Wop = np.ascontiguousarray(WoT.reshape(2, 128, HS).transpose(1, 0, 2).reshape(128, 2 * HS))

        # score coefficients, padded layout, 4 heads side by side
        A3 = np.zeros((3, HPC * cw_pad), np.float32)
        # scatter indices: per (pair, out half) [128, M] int16
        sidx = np.zeros((2, 2, 128, M), np.int16)
        for kk, k in enumerate(heads):
            order = order_all[b, k]
            ms = m_all[b, k][order]
            ss = s_all[b, k][order]
            base = kk * cw_pad
            for jb in range(NJB):
                lo, hi = ranges[jb]
                if hi <= lo:
                    continue
                t = ms[lo:hi] - (128.0 * jb + 64.0)
                s_ = ss[lo:hi]
                o = base + offs_pad[jb]
                A3[0, o:o + hi - lo] = s_
                A3[1, o:o + hi - lo] = -2.0 * s_ * t
                A3[2, o:o + hi - lo] = s_ * t * t
            pair, sub = kk // 2, kk % 2
            o64 = order.astype(np.int64)
            iA = np.where(o64 < M // 2, o64, -1).astype(np.int16)
            iB = np.where(o64 >= M // 2, o64 - M // 2, -1).astype(np.int16)
            rows = slice(64 * sub, 64 * sub + 64)
            sidx[pair, 0, rows, :] = iA[None, :]
            sidx[pair, 1, rows, :] = iB[None, :]

        u = np.arange(-64, 64, dtype=np.float32)
        b3 = np.stack([u * u, u, np.ones(128, np.float32)])
        basis = np.zeros((99, 128), np.float32)
        for rg in range(4):
            basis[32 * rg:32 * rg + 3] = b3

        in_maps.append({
            "hTp": hTp, "Wvp": Wvp, "Wop": Wop,
            "A3p": A3, "sidxp": sidx.reshape(512, M), "basis": basis,
        })

    return in_maps, tuple(ranges)


def _build_kernel(ranges):
    import concourse.tile as tile
    from concourse import bacc, mybir
    from concourse.alu_op_type import AluOpType

    F32 = mybir.dt.float32
    F32R = mybir.dt.float32r
    BF16 = mybir.dt.bfloat16
    I16 = mybir.dt.int16
    Exp = mybir.ActivationFunctionType.Exp

    wpads, offs_pad, cw_pad, groups = _schedule(ranges)

    nc = bacc.Bacc("TRN2", target_bir_lowering=False, debug=False, num_devices=NCORES)

    hTp = nc.dram_tensor("hTp", [128, NJB * 1024], BF16, kind="ExternalInput")
    Wvp = nc.dram_tensor("Wvp", [128, 8 * CP], BF16, kind="ExternalInput")
    Wop = nc.dram_tensor("Wop", [128, 2 * HS], BF16, kind="ExternalInput")
    A3p = nc.dram_tensor("A3p", [3, HPC * cw_pad], F32, kind="ExternalInput")
    sidxp = nc.dram_tensor("sidxp", [512, M], I16, kind="ExternalInput")
    basis = nc.dram_tensor("basis", [99, 128], F32, kind="ExternalInput")
    out_part = nc.dram_tensor("out_part", [M, HS], BF16, kind="ExternalOutput")
    if DEBUG_TAPS:
        dbg_at = nc.dram_tensor("dbg_at", [128, HPC * cw_pad], BF16, kind="ExternalOutput")
        dbg_v = nc.dram_tensor("dbg_v", [128, NJB * HPC * 2 * 64], BF16, kind="ExternalOutput")
        dbg_pair = nc.dram_tensor("dbg_pair", [128, 2 * M], BF16, kind="ExternalOutput")
        dbg_nat = nc.dram_tensor("dbg_nat", [128, 2 * M], BF16, kind="ExternalOutput")
        dbg_ops = nc.dram_tensor("dbg_ops", [128, HM], mybir.dt.float32, kind="ExternalOutput")

    cast_ctr = [0]

    with tile.TileContext(nc) as tc:
        with (
            tc.tile_pool(name="persist", bufs=1) as persist,
            tc.tile_pool(name="vpool", bufs=1) as vpool,
            tc.tile_pool(name="norm_pool", bufs=4) as norm_pool,
            tc.tile_pool(name="out_pool", bufs=3) as out_pool,
            tc.tile_pool(name="ps", bufs=2, space="PSUM") as ps,
        ):
            # Wv + first hT chunk are the first value matmul's only deps:
            # issue them first (DMA completion semaphores serialize, so every
            # extra DMA ahead of a consumer delays it)
            Wv_sb = persist.tile([128, 8 * CP], BF16, name="Wv")
            nc.sync.dma_start(Wv_sb[:], Wvp[:])
            hT_sb = persist.tile([128, NJB * 1024], BF16, name="hT")
            nc.sync.dma_start(hT_sb[:, 0:4096], hTp[:, 0:4096])
            basis_sb = persist.tile([99, 128], F32, name="basis")
            nc.sync.dma_start(basis_sb[:], basis[:])
            # A3 split per (head, rg): [3, n] DMAs are per-partition-line bound.
            # head 0 first, then the remaining hT chunks, then heads 1-3.
            A_t = persist.tile([99, HPC * cw_pad], F32, name="At")
            for rg in range(4):
                nc.sync.dma_start(A_t[32 * rg:32 * rg + 3, 0:cw_pad],
                                  A3p[:, 0:cw_pad])
            for hh in range(1, 4):
                nc.sync.dma_start(hT_sb[:, hh * 4096:(hh + 1) * 4096],
                                  hTp[:, hh * 4096:(hh + 1) * 4096])
            for kk in range(1, HPC):
                ks = slice(kk * cw_pad, (kk + 1) * cw_pad)
                for rg in range(4):
                    nc.sync.dma_start(A_t[32 * rg:32 * rg + 3, ks], A3p[:, ks])
            sidx_sb = [[persist.tile([128, M], I16, name=f"sidx{p}{hh}")
                        for hh in range(2)] for p in range(2)]
            for p in range(2):
                for hh in range(2):
                    nc.sync.dma_start(
                        sidx_sb[p][hh][:],
                        sidxp[(2 * p + hh) * 128:(2 * p + hh + 1) * 128, :])
            Wo_sb = persist.tile([128, 2 * HS], BF16, name="Wo")
            nc.sync.dma_start(Wo_sb[:], Wop[:])

            pair_sb = [persist.tile([128, M], BF16, name=f"pair{p}") for p in range(2)]
            nat_sb = [persist.tile([128, M], BF16, name=f"nat{p}") for p in range(2)]

            # value tiles: per block, [128 keys, 4 heads x (64 ones | 64 value)];
            # ones first so softmax denominators land at PSUM partitions 0..63
            # (custom-DVE reciprocal breaks on PSUM reads at partition base 64)
            v_sb = [vpool.tile([128, HPC, 2, 64], BF16, name=f"v{jb}") for jb in range(NJB)]
            for jb in range(NJB):
                nc.vector.memset(v_sb[jb][:, :, 0, :], 1.0)

            # attention-weight tiles, padded layout (junk in pad cols, never read)
            at_sb = [persist.tile([128, cw_pad], BF16, name=f"at{kk}") for kk in range(HPC)]

            value_done = [False] * NJB

            def emit_value(jb):
                if value_done[jb]:
                    return
                value_done[jb] = True
                pv = ps.tile([128, HPC, 64], F32, name="pv", tag="sc", bufs=4)
                for c in range(8):
                    nc.tensor.matmul(
                        pv[:], hT_sb[:, jb * 1024 + c * 128: jb * 1024 + c * 128 + 128],
                        Wv_sb[:, c * CP:(c + 1) * CP],
                        start=(c == 0), stop=(c == 7))
                nc.vector.tensor_copy(v_sb[jb][:, :, 1, :], pv[:])

            rg_ctr = [0]

            def emit_attn(kk, uh, o_ps):
                h_lo, h_hi = uh * HM, (uh + 1) * HM
                bank_first = [True, True]
                for jb in range(NJB):
                    lo, hi = ranges[jb]
                    ulo, uhi = max(lo, h_lo), min(hi, h_hi)
                    if uhi <= ulo:
                        continue
                    vblk = v_sb[jb][:].rearrange("p a b c -> p (a b c)")[
                        :, kk * 128:(kk + 1) * 128]
                    for q in range(2):
                        s0 = max(ulo, h_lo + q * 512)
                        s1 = min(uhi, h_lo + (q + 1) * 512)
                        if s1 <= s0:
                            continue
                        nc.tensor.matmul(
                            o_ps[:, s0 - h_lo:s1 - h_lo], vblk,
                            at_sb[kk][:, offs_pad[jb] + s0 - lo: offs_pad[jb] + s1 - lo],
                            start=bank_first[q], stop=False,
                            skip_group_check=True)
                        bank_first[q] = False

            def emit_norm(kk, uh, o_ps):
                pair, sub = kk // 2, kk % 2
                for q in range(2):
                    qs = slice(q * 512, (q + 1) * 512)
                    rcs = norm_pool.tile([64, 512], F32, name="rcs", tag="rcs")
                    nc.vector.reciprocal_approx_fast(rcs[:], o_ps[0:64, qs])
                    nc.vector.tensor_tensor(
                        pair_sb[pair][64 * sub:64 * sub + 64,
                                      uh * HM + q * 512: uh * HM + (q + 1) * 512],
                        o_ps[64:128, qs], rcs[:], AluOpType.mult)

            vjb = [0]  # next value block to emit
            while vjb[0] < 4:
                emit_value(vjb[0])
                vjb[0] += 1

            for kk in range(HPC):
                base = kk * cw_pad
                for gi, (jb0, njb, gw) in enumerate(groups):
                    rg = rg_ctr[0] % 4
                    rg_ctr[0] += 1
                    sc = ps.tile([128, 512], F32, name="sc", tag="sc", bufs=4)
                    nc.tensor.matmul(
                        sc[:, :gw], basis_sb[32 * rg:32 * rg + 3, :],
                        A_t[32 * rg:32 * rg + 3,
                            base + offs_pad[jb0]: base + offs_pad[jb0] + gw],
                        start=True, stop=True, tile_position=(32 * rg, 0))
                    nc.scalar.activation(
                        at_sb[kk][:, offs_pad[jb0]: offs_pad[jb0] + gw],
                        sc[:, :gw], Exp, scale=-1.0)
                    if kk == 0:
                        nv = (16 * (gi + 1)) // len(groups)
                        while vjb[0] < nv:
                            emit_value(vjb[0])
                            vjb[0] += 1
                for uh in range(2):
                    o_ps = ps.tile([128, HM], F32, name="oT", tag="oT", bufs=2)
                    emit_attn(kk, uh, o_ps)
                    if DEBUG_TAPS and kk == 0 and uh == 0:
                        dbg_ops_sb = persist.tile([128, HM], F32, name="dbgops")
                        nc.vector.tensor_copy(dbg_ops_sb[:], o_ps[:])
                        nc.sync.dma_start(dbg_ops[:], dbg_ops_sb[:])
                    emit_norm(kk, uh, o_ps)

            # un-permute sorted -> natural: local_scatter x4 back-to-back
            # (these pack; indirect_copy eats ~28us per launch regardless)
            for p in range(2):
                for hh in range(2):
                    nc.gpsimd.local_scatter(
                        nat_sb[p][:, hh * HM:(hh + 1) * HM], pair_sb[p][:],
                        sidx_sb[p][hh][:],
                        channels=128, num_elems=HM, num_idxs=M)

            # ---- output projection ----
            for ic in range(M // 128):
                ics = slice(ic * 128, (ic + 1) * 128)
                ot = out_pool.tile([128, HS], BF16, name="ot", tag="ot")
                for jh in range(2):
                    jhs = slice(jh * 512, (jh + 1) * 512)
                    pp = ps.tile([128, 512], F32, name="pp", tag="sc", bufs=4)
                    nc.tensor.matmul(pp[:], nat_sb[0][:, ics],
                                     Wo_sb[:, jh * 512: jh * 512 + 512],
                                     start=True, stop=False)
                    nc.tensor.matmul(pp[:], nat_sb[1][:, ics],
                                     Wo_sb[:, HS + jh * 512: HS + jh * 512 + 512],
                                     start=False, stop=True)
                    if cast_ctr[0] % 2 == 0:
                        nc.vector.tensor_copy(ot[:, jhs], pp[:])
                    else:
                        nc.scalar.copy(ot[:, jhs], pp[:])
                    cast_ctr[0] += 1
                nc.sync.dma_start(out_part[ics, :], ot[:])

    nc.compile()
    return nc


def kernel(h, W_span, W_val, W_out):
    _ensure_ntff_hook()
    from concourse.bass_utils import run_bass_kernel_spmd

    in_maps, ranges = _build_host_data(h, W_span, W_val, W_out)
    nc = _CACHE.get(ranges)
    if nc is None:
        nc = _build_kernel(ranges)
        _CACHE[ranges] = nc

    res = run_bass_kernel_spmd(nc, in_maps, list(range(NCORES)), trace=False)

    out = np.zeros((B, M, HS), np.float32)
    for core in range(NCORES):
        out[core // HPC] += res.results[core]["out_part"].astype(np.float32)
    return out
